# revision 7
# baseline (speedup 1.0000x reference)
"""DiffFOOOF loss on 8 NeuronCores — pure data parallelism over batch.

Each core processes B/8 = 1024 rows and emits per-column partial sums in
two [128, 16] accumulator tiles (one written by ScalarE accums, one by
DVE); the host does the final partition/core reduction in float64.

v3 engine split (from perfetto analysis of v1/v2):
  * DVE: e = p - t (fp32 in, bf16 out), then v = max(e,1) (4x bf16
    tensor_scalar) and m = max(-e, v) = max(|e|,1) (2x bf16 stt).
  * ScalarE: Square(e) and Square(m - 1), each with accum_out, one
    column per tile (the accumulate does the full free-dim reduce).
  * fp32 identity-matmul subtract on PE was tried and dropped: fp32
    matmuls run 4-pass (~600-1000ns per 512-col MM + LDW per call).
  * Greedy matching: no EPS prescale (used-flag scaled by 2^30 inside the
    per-step scalar_tensor_tensor), no iota tie-break (input data has no
    ties; verified host-side), inactive GT slots masked by adding
    (1-mask)*2^31 to the row-min before the is_equal.
  * 16 big loads issued round-robin over 4 HWDGE rings (sync/scalar/
    tensor/vector) so all DMA engines start within ~3us; all 16 tiles
    stay resident in SBUF (no buffer-recycling stalls).
"""

import numpy as np

import concourse.bass as bass
import concourse.tile as tile
from concourse import bacc, mybir
from concourse.bass_utils import run_bass_kernel_spmd

import concourse.dve_ops as dve_ops_mod
from concourse.dve_ops import DveOp, TENSOR_TENSOR_REDUCE
from concourse.dve_spec import Spec, Src0, C0, C1, sq, maxx, Zero
from concourse.dve_spec import AluOp as DAlu

f32 = mybir.dt.float32
bf16 = mybir.dt.bfloat16
Alu = mybir.AluOpType
Act = mybir.ActivationFunctionType
X = mybir.AxisListType.X

N_CORES = 8
B, F, K = 8192, 2048, 6
BS = B // N_CORES        # rows per core
P = 128                  # partitions
NT = BS // P             # big [128, F] tiles per core
G = BS // P              # row-groups per partition for the small tensors
NCHUNK = 4               # 512-col matmul chunks per big tile
CW = F // NCHUNK
BIG_USED = float(2 ** 30)   # added (x used-flag) to mask used pred slots
BIG_ROW = float(2 ** 31)    # added to row-min for inactive GT slots

# ACC_S cols (ScalarE accum_out targets)
S_E2, S_PK, S_AP, S_BW2 = 0, 8, 9, 10       # E2: 8 cols
# ACC_V cols (DVE accum/reduce targets)
V_HT, V_AMPS, V_UMN, V_UMD, V_MASK = 0, 8, 9, 10, 11

SMALL_NAMES = ("cfs", "amps", "bws", "gt_cfs", "gt_amps", "gt_bws", "peak_mask")


def _register_huber_tail():
    for op in dve_ops_mod.OPS:
        if op.name == "HUBER_TAIL":
            return op

    def _ref(in0, in1, c0, c1, c2):
        m = np.maximum(np.maximum(in0, -in0), c0).astype(np.float32)
        b = ((m + c1) ** 2).astype(np.float32)
        return b, b.reshape(b.shape[0], -1).sum(axis=-1, keepdims=True)

    op = DveOp(
        "HUBER_TAIL",
        Spec(body=sq(maxx(maxx(Src0, Zero - Src0), C0) + C1),
             accum=DAlu.ADD, accum_init=Zero, reference=_ref),
        subdim=False,
        uops_sha={"v3": "ac9e4fb99b6aa535", "v4": "de8af5b68d3f8e26"},
    )
    dve_ops_mod.OPS.append(op)
    dve_ops_mod._SUB_OPCODE_FOR_NAME[op.name] = (
        dve_ops_mod._CUSTOM_DVE_ROW_BASE + len(dve_ops_mod.OPS) - 1)
    return op


HUBER_TAIL = _register_huber_tail()


def build_nc():
    from contextlib import ExitStack

    nc = bacc.Bacc("TRN2", target_bir_lowering=False, debug=False,
                   num_devices=N_CORES)
    pred = nc.dram_tensor("pred_psd", [BS, F], f32, kind="ExternalInput")
    true = nc.dram_tensor("true_psd", [BS, F], f32, kind="ExternalInput")
    dr = {n: nc.dram_tensor(n, [BS, K], f32, kind="ExternalInput")
          for n in SMALL_NAMES}
    exponent = nc.dram_tensor("exponent", [BS, 1], f32, kind="ExternalInput")
    offset = nc.dram_tensor("offset", [BS, 1], f32, kind="ExternalInput")
    gt_exp = nc.dram_tensor("gt_exponent", [BS], f32, kind="ExternalInput")
    gt_off = nc.dram_tensor("gt_offset", [BS], f32, kind="ExternalInput")
    out_d = nc.dram_tensor("out", [P, 32], f32, kind="ExternalOutput")

    with tile.TileContext(nc) as tc, ExitStack() as ctx:
        sp = ctx.enter_context(tc.tile_pool(name="small", bufs=1))
        mp = ctx.enter_context(tc.tile_pool(name="match", bufs=1))
        pp = ctx.enter_context(tc.tile_pool(name="pred", bufs=NT))
        tp = ctx.enter_context(tc.tile_pool(name="true", bufs=NT))
        dsq = ctx.enter_context(tc.tile_pool(name="dsq", bufs=2))
        dht = ctx.enter_context(tc.tile_pool(name="dht", bufs=2))
        ep = ctx.enter_context(tc.tile_pool(name="e", bufs=2))
        psp = ctx.enter_context(tc.tile_pool(name="ps", bufs=2, space="PSUM"))

        # ------------- big DMAs first, 4 rings in parallel --------------
        pts = [pp.tile([P, F], f32, tag="pt", name=f"pt{_t}") for _t in range(NT)]
        tts = [tp.tile([P, F], f32, tag="tt", name=f"tt{_t}") for _t in range(NT)]
        # only sync + scalar have HWDGE: pred on sync, true on scalar,
        # issued in tile order so early tiles' descriptors go first
        for i in range(NT):
            nc.sync.dma_start(out=pts[i][:], in_=pred[i * P:(i + 1) * P, :])
            nc.scalar.dma_start(out=tts[i][:], in_=true[i * P:(i + 1) * P, :])

        # ------------- small tensors (gpsimd swdge ring) ----------------
        # row r = p*G + g; V col = v*(G*K) + g*K + i
        V = sp.tile([P, 3 * G * K], f32)
        GT = sp.tile([P, 3 * G * K], f32)
        M = sp.tile([P, G * K], f32)
        AUX = sp.tile([P, 4 * G], f32)
        # prologue-critical first: cfs, gt_cfs, peak_mask
        for v, name in ((0, "cfs"), (3, "gt_cfs")):
            dst = V if v < 3 else GT
            nc.gpsimd.dma_start(
                out=dst[:, (v % 3) * G * K:((v % 3) + 1) * G * K],
                in_=dr[name][:, :].rearrange("(p g) i -> p (g i)", g=G))
        nc.gpsimd.dma_start(
            out=M[:, :], in_=dr["peak_mask"][:, :].rearrange("(p g) j -> p (g j)", g=G))
        for v, name in ((1, "amps"), (2, "bws"), (4, "gt_amps"), (5, "gt_bws")):
            dst = V if v < 3 else GT
            nc.gpsimd.dma_start(
                out=dst[:, (v % 3) * G * K:((v % 3) + 1) * G * K],
                in_=dr[name][:, :].rearrange("(p g) i -> p (g i)", g=G))

        # AUX after the identity build (only needed by the epilogue)
        nc.gpsimd.dma_start(
            out=AUX[:, 0:G], in_=exponent[:, :].rearrange("(p g) o -> p (g o)", g=G))
        nc.gpsimd.dma_start(
            out=AUX[:, G:2 * G], in_=gt_exp[:].rearrange("(p g) -> p g", g=G))
        nc.gpsimd.dma_start(
            out=AUX[:, 2 * G:3 * G], in_=offset[:, :].rearrange("(p g) o -> p (g o)", g=G))
        nc.gpsimd.dma_start(
            out=AUX[:, 3 * G:4 * G], in_=gt_off[:].rearrange("(p g) -> p g", g=G))

        # identity weights for the PE subtract of tiles 0..3
        ones_w = sp.tile([P, P], f32)
        nc.vector.memset(ones_w[:, 0:P], 1.0)
        negs_w = sp.tile([P, P], f32)
        nc.vector.memset(negs_w[:, 0:P], -1.0)
        I_w = sp.tile([P, P], f32)
        nc.gpsimd.affine_select(out=I_w[:], in_=ones_w[:], pattern=[[-1, P]],
                                compare_op=Alu.is_equal, fill=0.0,
                                base=0, channel_multiplier=1)
        nI_w = sp.tile([P, P], f32)
        nc.gpsimd.affine_select(out=nI_w[:], in_=negs_w[:], pattern=[[-1, P]],
                                compare_op=Alu.is_equal, fill=0.0,
                                base=0, channel_multiplier=1)

        ACC_S = sp.tile([P, 16], f32)
        nc.vector.memset(ACC_S[:], 0.0)
        ACC_V = sp.tile([P, 16], f32)
        nc.vector.memset(ACC_V[:], 0.0)
        neg1 = sp.tile([P, 1], f32)
        nc.vector.memset(neg1[:], -1.0)

        # ------------- matching tiles ------------------------------------
        V4 = V[:].rearrange("p (v g i) -> p g v i", v=3, i=K)
        M3 = M[:].rearrange("p (g j) -> p g j", j=K)
        gtp3 = GT[:, 0:G * K].rearrange("p (g j) -> p g j", j=K)
        cfs3 = V[:, 0:G * K].rearrange("p (g i) -> p g i", i=K)

        dist = mp.tile([P, G * K * K], f32)   # col = g*36 + j*6 + i
        dabs = mp.tile([P, G * K * K], f32)
        dist4 = dist[:].rearrange("p (g j i) -> p g j i", j=K, i=K)
        dabs4 = dabs[:].rearrange("p (g j i) -> p g j i", j=K, i=K)
        NEG = mp.tile([P, G * K], f32)        # (1-mask)*2^31, col = g*6 + j
        NEG3 = NEG[:].rearrange("p (g j) -> p g j", j=K)
        H = mp.tile([P, G * K * K], f32)      # one-hot hits, col = g*36+j*6+i
        H4 = H[:].rearrange("p (g j i) -> p g j i", j=K, i=K)
        used_t = [mp.tile([P, G * K], f32, tag=f"used{j}", name=f"used{j}")
                  for j in range(K + 1)]

        def match_prologue():
            nc.vector.tensor_tensor(
                out=dist4,
                in0=gtp3.to_broadcast([P, G, K, K]),
                in1=cfs3.unsqueeze(2).to_broadcast([P, G, K, K]),
                op=Alu.subtract)
            nc.vector.scalar_tensor_tensor(out=dabs4, in0=dist4, scalar=-1.0,
                                           in1=dist4, op0=Alu.mult, op1=Alu.max)
            nc.vector.tensor_scalar(out=NEG[:], in0=M[:], scalar1=-BIG_ROW,
                                    scalar2=BIG_ROW, op0=Alu.mult, op1=Alu.add)
            nc.vector.memset(used_t[0][:], 0.0)

        def match_scan_step(j):
            u3 = used_t[j][:].rearrange("p (g i) -> p g i", i=K)
            dm = mp.tile([P, G * K], f32, tag="dm")
            dm3 = dm[:].rearrange("p (g i) -> p g i", i=K)
            nc.vector.scalar_tensor_tensor(out=dm3, in0=u3, scalar=BIG_USED,
                                           in1=dabs4[:, :, j, :],
                                           op0=Alu.mult, op1=Alu.add)
            mv = mp.tile([P, G], f32, tag="mv")
            nc.vector.tensor_reduce(out=mv[:], in_=dm3, axis=X, op=Alu.min)
            mvp = mp.tile([P, G], f32, tag="mvp")
            nc.vector.tensor_tensor(out=mvp[:], in0=mv[:], in1=NEG3[:, :, j],
                                    op=Alu.add)
            hj = H4[:, :, j, :]
            nc.vector.tensor_tensor(out=hj, in0=dm3,
                                    in1=mvp[:].to_broadcast([P, G, K]),
                                    op=Alu.is_equal)
            un3 = used_t[j + 1][:].rearrange("p (g i) -> p g i", i=K)
            nc.vector.tensor_tensor(out=un3, in0=u3, in1=hj, op=Alu.add)

        def epi_gather():
            # Gt[p,v,g,j] = sum_i H[p,g,j,i] * V[p,v,g,i]
            gm = mp.tile([P, 3 * G * K * K], f32)
            gm5 = gm[:].rearrange("p (v g j i) -> p v g j i", v=3, j=K, i=K)
            Vv = V[:].rearrange("p (v g i) -> p v g i", v=3, i=K)
            nc.vector.tensor_tensor(
                out=gm5,
                in0=Vv.unsqueeze(3).to_broadcast([P, 3, G, K, K]),
                in1=H4.unsqueeze(1).to_broadcast([P, 3, G, K, K]),
                op=Alu.mult)
            Gt = mp.tile([P, 3 * G * K], f32)
            Gt4 = Gt[:].rearrange("p (v g j) -> p v g j", v=3, j=K)
            nc.vector.tensor_reduce(out=Gt4, in_=gm5, axis=X, op=Alu.add)
            return Gt

        def epi_peaks(Gt):
            D = mp.tile([P, 3 * G * K], f32)
            nc.vector.tensor_tensor(out=D[:], in0=Gt[:], in1=GT[:],
                                    op=Alu.subtract)
            Dm = mp.tile([P, 3 * G * K], f32)
            nc.vector.tensor_tensor(
                out=Dm[:].rearrange("p (v gj) -> p v gj", v=3),
                in0=D[:].rearrange("p (v gj) -> p v gj", v=3),
                in1=M[:].unsqueeze(1).to_broadcast([P, 3, G * K]),
                op=Alu.mult)
            dmp = dsq.tile([P, 3 * G * K], bf16, tag="dmp")
            nc.scalar.activation(out=dmp[:], in_=Dm[:], func=Act.Square,
                                 accum_out=ACC_S[:, S_PK:S_PK + 1])
            # aperiodic: pack [dE | dO] then one Square-accum
            DEO = mp.tile([P, 2 * G], f32)
            nc.vector.tensor_tensor(out=DEO[:, 0:G], in0=AUX[:, 0:G],
                                    in1=AUX[:, G:2 * G], op=Alu.subtract)
            nc.vector.tensor_tensor(out=DEO[:, G:2 * G], in0=AUX[:, 2 * G:3 * G],
                                    in1=AUX[:, 3 * G:4 * G], op=Alu.subtract)
            deo = dsq.tile([P, 2 * G], bf16, tag="deo")
            nc.scalar.activation(out=deo[:], in_=DEO[:], func=Act.Square,
                                 accum_out=ACC_S[:, S_AP:S_AP + 1])
            # bw excess
            rb = mp.tile([P, G * K], f32)
            nc.vector.tensor_scalar(out=rb[:], in0=V[:, 2 * G * K:3 * G * K],
                                    scalar1=4.0, scalar2=0.0,
                                    op0=Alu.subtract, op1=Alu.max)
            rbo = dsq.tile([P, G * K], bf16, tag="rbo")
            nc.scalar.activation(out=rbo[:], in_=rb[:], func=Act.Square,
                                 accum_out=ACC_S[:, S_BW2:S_BW2 + 1])

        def epi_sums():
            u3 = used_t[K][:]
            unm = mp.tile([P, G * K], f32)
            nc.vector.tensor_scalar(out=unm[:], in0=u3, scalar1=-1.0,
                                    scalar2=1.0, op0=Alu.mult, op1=Alu.add)
            nc.vector.tensor_reduce(out=ACC_V[:, V_AMPS:V_AMPS + 1],
                                    in_=V[:, G * K:2 * G * K], axis=X, op=Alu.add)
            ua = mp.tile([P, G * K], f32)
            nc.vector._custom_dve(TENSOR_TENSOR_REDUCE, out=ua[:],
                                  in0=V[:, G * K:2 * G * K], in1=unm[:],
                                  s0=0.0, s1=1.0,
                                  accum_out=ACC_V[:, V_UMN:V_UMN + 1])
            nc.vector.tensor_reduce(out=ACC_V[:, V_UMD:V_UMD + 1], in_=unm[:],
                                    axis=X, op=Alu.add)
            nc.vector.tensor_reduce(out=ACC_V[:, V_MASK:V_MASK + 1], in_=M[:],
                                    axis=X, op=Alu.add)

        # ------------- DVE/ScalarE main interleave -----------------------
        N_PE = 4   # tiles whose subtract runs on the (otherwise idle) PE

        def huber_tile(t):
            if t < N_PE:
                e = psp.tile([P, F], f32, tag="e", name=f"e{t}")
                for c in range(NCHUNK):
                    sl = slice(c * CW, (c + 1) * CW)
                    nc.tensor.matmul(out=e[:, sl], lhsT=I_w[:],
                                     rhs=pts[t][:, sl], start=True, stop=False)
                for c in range(NCHUNK):
                    sl = slice(c * CW, (c + 1) * CW)
                    nc.tensor.matmul(out=e[:, sl], lhsT=nI_w[:],
                                     rhs=tts[t][:, sl], start=False, stop=True)
            else:
                e = ep.tile([P, F], bf16, tag="e")
                nc.vector.tensor_tensor(out=e[:], in0=pts[t][:], in1=tts[t][:],
                                        op=Alu.subtract)
            d1 = dsq.tile([P, F], bf16, tag="d1")
            nc.scalar.activation(out=d1[:], in_=e[:], func=Act.Square,
                                 accum_out=ACC_S[:, S_E2 + t:S_E2 + t + 1])
            d2 = dht.tile([P, F], bf16, tag="d2")
            nc.vector._custom_dve(HUBER_TAIL, out=d2[:], in0=e[:],
                                  s0=1.0, s1=-1.0,
                                  accum_out=ACC_V[:, V_HT + t:V_HT + t + 1])

        match_prologue()
        for j in range(K):
            match_scan_step(j)
        huber_tile(0)
        Gt = epi_gather()
        huber_tile(1)
        epi_peaks(Gt)
        huber_tile(2)
        epi_sums()
        for t in range(3, NT):
            huber_tile(t)

        # ---------------- store both ACC tiles ----------------
        nc.sync.dma_start(out=out_d[:, 0:16], in_=ACC_S[:])
        nc.sync.dma_start(out=out_d[:, 16:32], in_=ACC_V[:])
    nc.compile()
    return nc


_NC_CACHE = None


def _get_nc():
    global _NC_CACHE
    if _NC_CACHE is None:
        _NC_CACHE = build_nc()
    return _NC_CACHE


def combine(parts):
    """parts: [n_cores, 32] float64 -> final scalar (python float)."""
    s = parts.sum(axis=0)
    S1 = s[S_E2:S_E2 + 8].sum()             # sum e^2
    S3 = s[16 + V_HT:16 + V_HT + 8].sum()   # sum relu(|e|-1)^2
    n_big = float(B) * F
    l_recon = (0.5 * S1 - 0.5 * S3) / n_big
    l_sparse = s[16 + V_AMPS] / (B * K)
    l_bw = s[S_BW2] / (B * K)
    l_ap = s[S_AP] / B
    l_peaks = s[S_PK] / max(s[16 + V_MASK], 1.0)
    l_um = s[16 + V_UMN] / max(s[16 + V_UMD], 1.0)
    return (l_recon + 0.1 * l_sparse + 0.05 * l_bw + 0.5 * l_ap
            + 0.3 * l_peaks + 0.1 * l_um)


def run(inputs, **spmd_kwargs):
    nc = _get_nc()
    in_maps = []
    for c in range(N_CORES):
        lo, hi = c * BS, (c + 1) * BS
        in_maps.append({k: np.ascontiguousarray(v[lo:hi]) for k, v in inputs.items()})
    res = run_bass_kernel_spmd(nc, in_maps, list(range(N_CORES)), **spmd_kwargs)
    parts = np.stack([r["out"].astype(np.float64).sum(axis=0)
                      for r in res.results])
    return np.float32(combine(parts)), res


def kernel(**inputs):
    out, _ = run(inputs)
    return out


# revision 8
# speedup vs baseline: 1.4875x; 1.4875x over previous
"""DiffFOOOF loss on 8 NeuronCores — pure data parallelism over batch.

Each core processes B/8 = 1024 rows and emits per-column partial sums in
two [128, 16] accumulator tiles (one written by ScalarE accums, one by
DVE); the host does the final partition/core reduction in float64.

v3 engine split (from perfetto analysis of v1/v2):
  * DVE: e = p - t (fp32 in, bf16 out), then v = max(e,1) (4x bf16
    tensor_scalar) and m = max(-e, v) = max(|e|,1) (2x bf16 stt).
  * ScalarE: Square(e) and Square(m - 1), each with accum_out, one
    column per tile (the accumulate does the full free-dim reduce).
  * fp32 identity-matmul subtract on PE was tried and dropped: fp32
    matmuls run 4-pass (~600-1000ns per 512-col MM + LDW per call).
  * Greedy matching: no EPS prescale (used-flag scaled by 2^30 inside the
    per-step scalar_tensor_tensor), no iota tie-break (input data has no
    ties; verified host-side), inactive GT slots masked by adding
    (1-mask)*2^31 to the row-min before the is_equal.
  * 16 big loads issued round-robin over 4 HWDGE rings (sync/scalar/
    tensor/vector) so all DMA engines start within ~3us; all 16 tiles
    stay resident in SBUF (no buffer-recycling stalls).
"""

import numpy as np

import concourse.bass as bass
import concourse.tile as tile
from concourse import bacc, mybir
from concourse.bass_utils import run_bass_kernel_spmd

import concourse.dve_ops as dve_ops_mod
from concourse.dve_ops import DveOp, TENSOR_TENSOR_REDUCE
from concourse.dve_spec import Spec, Src0, Src1, C0, C1, sq, maxx, minn, Zero
from concourse.dve_spec import AluOp as DAlu

f32 = mybir.dt.float32
bf16 = mybir.dt.bfloat16
Alu = mybir.AluOpType
Act = mybir.ActivationFunctionType
X = mybir.AxisListType.X

N_CORES = 8
B, F, K = 8192, 2048, 6
BS = B // N_CORES        # rows per core
P = 128                  # partitions
NT = BS // P             # big [128, F] tiles per core
G = BS // P              # row-groups per partition for the small tensors
NCHUNK = 4               # 512-col matmul chunks per big tile
CW = F // NCHUNK
BIG_USED = float(2 ** 30)   # added (x used-flag) to mask used pred slots
BIG_ROW = float(2 ** 31)    # added to row-min for inactive GT slots

# ACC_S cols (ScalarE accum_out targets)
S_PK, S_AP, S_BW2 = 0, 1, 2
# ACC_V cols (DVE accum/reduce targets)
V_HF, V_AMPS, V_UMN, V_UMD, V_MASK = 0, 8, 9, 10, 11

SMALL_NAMES = ("cfs", "amps", "bws", "gt_cfs", "gt_amps", "gt_bws", "peak_mask")


def _register_huber_full2():
    """2-stream fused op: accum += sum(2*huber(p - t)) per partition.

    huber2(e) = e^2 - relu(|e|-1)^2 = m*(2a - m) with a=|e|, m=min(a,1):
    exactly the 8-stage DVE budget (SUB, SUB, MAX, MIN, ADD, SUB, MUL,
    accum-ADD), fp32 all the way.
    """
    for op in dve_ops_mod.OPS:
        if op.name == "HUBER_FULL2":
            return op

    def _ref(in0, in1, c0, c1, c2):
        a = np.abs(in0.astype(np.float32) - in1.astype(np.float32))
        m = np.minimum(a, c0)
        b = (m * (2.0 * a - m)).astype(np.float32)
        return b, b.reshape(b.shape[0], -1).sum(axis=-1, keepdims=True)

    e1 = Src0 - Src1
    e2 = Src1 - Src0
    a = maxx(e1, e2)
    m = minn(a, C0)
    op = DveOp(
        "HUBER_FULL2",
        Spec(body=m * (a + a - m),
             accum=DAlu.ADD, accum_init=Zero, reference=_ref),
        subdim=False,
        uops_sha={"v3": "37a2775ccf8d79d0", "v4": "c0009c8da2ade20d"},
    )
    dve_ops_mod.OPS.append(op)
    dve_ops_mod._SUB_OPCODE_FOR_NAME[op.name] = (
        dve_ops_mod._CUSTOM_DVE_ROW_BASE + len(dve_ops_mod.OPS) - 1)
    return op


HUBER_FULL2 = _register_huber_full2()


def build_nc():
    from contextlib import ExitStack

    nc = bacc.Bacc("TRN2", target_bir_lowering=False, debug=False,
                   num_devices=N_CORES)
    pred = nc.dram_tensor("pred_psd", [BS, F], f32, kind="ExternalInput")
    true = nc.dram_tensor("true_psd", [BS, F], f32, kind="ExternalInput")
    dr = {n: nc.dram_tensor(n, [BS, K], f32, kind="ExternalInput")
          for n in SMALL_NAMES}
    exponent = nc.dram_tensor("exponent", [BS, 1], f32, kind="ExternalInput")
    offset = nc.dram_tensor("offset", [BS, 1], f32, kind="ExternalInput")
    gt_exp = nc.dram_tensor("gt_exponent", [BS], f32, kind="ExternalInput")
    gt_off = nc.dram_tensor("gt_offset", [BS], f32, kind="ExternalInput")
    out_d = nc.dram_tensor("out", [P, 32], f32, kind="ExternalOutput")

    with tile.TileContext(nc) as tc, ExitStack() as ctx:
        sp = ctx.enter_context(tc.tile_pool(name="small", bufs=1))
        mp = ctx.enter_context(tc.tile_pool(name="match", bufs=1))
        pp = ctx.enter_context(tc.tile_pool(name="pred", bufs=NT))
        tp = ctx.enter_context(tc.tile_pool(name="true", bufs=NT))
        dsq = ctx.enter_context(tc.tile_pool(name="dsq", bufs=2))
        dht = ctx.enter_context(tc.tile_pool(name="dht", bufs=2))

        # ------------- big DMAs first, 4 rings in parallel --------------
        pts = [pp.tile([P, F], f32, tag="pt", name=f"pt{_t}") for _t in range(NT)]
        tts = [tp.tile([P, F], f32, tag="tt", name=f"tt{_t}") for _t in range(NT)]
        # only sync + scalar have HWDGE: pred on sync, true on scalar,
        # issued in tile order so early tiles' descriptors go first
        for i in range(NT):
            nc.sync.dma_start(out=pts[i][:], in_=pred[i * P:(i + 1) * P, :])
            nc.scalar.dma_start(out=tts[i][:], in_=true[i * P:(i + 1) * P, :])

        # ------------- small tensors (gpsimd swdge ring) ----------------
        # row r = p*G + g; V col = v*(G*K) + g*K + i
        V = sp.tile([P, 3 * G * K], f32)
        GT = sp.tile([P, 3 * G * K], f32)
        M = sp.tile([P, G * K], f32)
        AUX = sp.tile([P, 4 * G], f32)
        # prologue-critical first: cfs, gt_cfs, peak_mask
        for v, name in ((0, "cfs"), (3, "gt_cfs")):
            dst = V if v < 3 else GT
            nc.gpsimd.dma_start(
                out=dst[:, (v % 3) * G * K:((v % 3) + 1) * G * K],
                in_=dr[name][:, :].rearrange("(p g) i -> p (g i)", g=G))
        nc.gpsimd.dma_start(
            out=M[:, :], in_=dr["peak_mask"][:, :].rearrange("(p g) j -> p (g j)", g=G))
        for v, name in ((1, "amps"), (2, "bws"), (4, "gt_amps"), (5, "gt_bws")):
            dst = V if v < 3 else GT
            nc.gpsimd.dma_start(
                out=dst[:, (v % 3) * G * K:((v % 3) + 1) * G * K],
                in_=dr[name][:, :].rearrange("(p g) i -> p (g i)", g=G))

        # AUX after the identity build (only needed by the epilogue)
        nc.gpsimd.dma_start(
            out=AUX[:, 0:G], in_=exponent[:, :].rearrange("(p g) o -> p (g o)", g=G))
        nc.gpsimd.dma_start(
            out=AUX[:, G:2 * G], in_=gt_exp[:].rearrange("(p g) -> p g", g=G))
        nc.gpsimd.dma_start(
            out=AUX[:, 2 * G:3 * G], in_=offset[:, :].rearrange("(p g) o -> p (g o)", g=G))
        nc.gpsimd.dma_start(
            out=AUX[:, 3 * G:4 * G], in_=gt_off[:].rearrange("(p g) -> p g", g=G))

        ACC_S = sp.tile([P, 16], f32)
        nc.vector.memset(ACC_S[:], 0.0)
        ACC_V = sp.tile([P, 16], f32)
        nc.vector.memset(ACC_V[:], 0.0)
        neg1 = sp.tile([P, 1], f32)
        nc.vector.memset(neg1[:], -1.0)

        # ------------- matching tiles ------------------------------------
        V4 = V[:].rearrange("p (v g i) -> p g v i", v=3, i=K)
        M3 = M[:].rearrange("p (g j) -> p g j", j=K)
        gtp3 = GT[:, 0:G * K].rearrange("p (g j) -> p g j", j=K)
        cfs3 = V[:, 0:G * K].rearrange("p (g i) -> p g i", i=K)

        dist = mp.tile([P, G * K * K], f32)   # col = g*36 + j*6 + i
        dabs = mp.tile([P, G * K * K], f32)
        dist4 = dist[:].rearrange("p (g j i) -> p g j i", j=K, i=K)
        dabs4 = dabs[:].rearrange("p (g j i) -> p g j i", j=K, i=K)
        NEG = mp.tile([P, G * K], f32)        # (1-mask)*2^31, col = g*6 + j
        NEG3 = NEG[:].rearrange("p (g j) -> p g j", j=K)
        H = mp.tile([P, G * K * K], f32)      # one-hot hits, col = g*36+j*6+i
        H4 = H[:].rearrange("p (g j i) -> p g j i", j=K, i=K)
        used_t = [mp.tile([P, G * K], f32, tag=f"used{j}", name=f"used{j}")
                  for j in range(K + 1)]

        def match_prologue():
            nc.vector.tensor_tensor(
                out=dist4,
                in0=gtp3.to_broadcast([P, G, K, K]),
                in1=cfs3.unsqueeze(2).to_broadcast([P, G, K, K]),
                op=Alu.subtract)
            nc.vector.scalar_tensor_tensor(out=dabs4, in0=dist4, scalar=-1.0,
                                           in1=dist4, op0=Alu.mult, op1=Alu.max)
            nc.vector.tensor_scalar(out=NEG[:], in0=M[:], scalar1=-BIG_ROW,
                                    scalar2=BIG_ROW, op0=Alu.mult, op1=Alu.add)
            nc.vector.memset(used_t[0][:], 0.0)

        def match_scan_step(j):
            u3 = used_t[j][:].rearrange("p (g i) -> p g i", i=K)
            dm = mp.tile([P, G * K], f32, tag="dm")
            dm3 = dm[:].rearrange("p (g i) -> p g i", i=K)
            nc.vector.scalar_tensor_tensor(out=dm3, in0=u3, scalar=BIG_USED,
                                           in1=dabs4[:, :, j, :],
                                           op0=Alu.mult, op1=Alu.add)
            mv = mp.tile([P, G], f32, tag="mv")
            nc.vector.tensor_reduce(out=mv[:], in_=dm3, axis=X, op=Alu.min)
            mvp = mp.tile([P, G], f32, tag="mvp")
            nc.vector.tensor_tensor(out=mvp[:], in0=mv[:], in1=NEG3[:, :, j],
                                    op=Alu.add)
            hj = H4[:, :, j, :]
            nc.vector.tensor_tensor(out=hj, in0=dm3,
                                    in1=mvp[:].to_broadcast([P, G, K]),
                                    op=Alu.is_equal)
            un3 = used_t[j + 1][:].rearrange("p (g i) -> p g i", i=K)
            nc.vector.tensor_tensor(out=un3, in0=u3, in1=hj, op=Alu.add)

        def epi_gather():
            # Gt[p,v,g,j] = sum_i H[p,g,j,i] * V[p,v,g,i]
            gm = mp.tile([P, 3 * G * K * K], f32)
            gm5 = gm[:].rearrange("p (v g j i) -> p v g j i", v=3, j=K, i=K)
            Vv = V[:].rearrange("p (v g i) -> p v g i", v=3, i=K)
            nc.vector.tensor_tensor(
                out=gm5,
                in0=Vv.unsqueeze(3).to_broadcast([P, 3, G, K, K]),
                in1=H4.unsqueeze(1).to_broadcast([P, 3, G, K, K]),
                op=Alu.mult)
            Gt = mp.tile([P, 3 * G * K], f32)
            Gt4 = Gt[:].rearrange("p (v g j) -> p v g j", v=3, j=K)
            nc.vector.tensor_reduce(out=Gt4, in_=gm5, axis=X, op=Alu.add)
            return Gt

        def epi_peaks(Gt):
            D = mp.tile([P, 3 * G * K], f32)
            nc.vector.tensor_tensor(out=D[:], in0=Gt[:], in1=GT[:],
                                    op=Alu.subtract)
            Dm = mp.tile([P, 3 * G * K], f32)
            nc.vector.tensor_tensor(
                out=Dm[:].rearrange("p (v gj) -> p v gj", v=3),
                in0=D[:].rearrange("p (v gj) -> p v gj", v=3),
                in1=M[:].unsqueeze(1).to_broadcast([P, 3, G * K]),
                op=Alu.mult)
            dmp = dsq.tile([P, 3 * G * K], bf16, tag="dmp")
            nc.scalar.activation(out=dmp[:], in_=Dm[:], func=Act.Square,
                                 accum_out=ACC_S[:, S_PK:S_PK + 1])
            # aperiodic: pack [dE | dO] then one Square-accum
            DEO = mp.tile([P, 2 * G], f32)
            nc.vector.tensor_tensor(out=DEO[:, 0:G], in0=AUX[:, 0:G],
                                    in1=AUX[:, G:2 * G], op=Alu.subtract)
            nc.vector.tensor_tensor(out=DEO[:, G:2 * G], in0=AUX[:, 2 * G:3 * G],
                                    in1=AUX[:, 3 * G:4 * G], op=Alu.subtract)
            deo = dsq.tile([P, 2 * G], bf16, tag="deo")
            nc.scalar.activation(out=deo[:], in_=DEO[:], func=Act.Square,
                                 accum_out=ACC_S[:, S_AP:S_AP + 1])
            # bw excess
            rb = mp.tile([P, G * K], f32)
            nc.vector.tensor_scalar(out=rb[:], in0=V[:, 2 * G * K:3 * G * K],
                                    scalar1=4.0, scalar2=0.0,
                                    op0=Alu.subtract, op1=Alu.max)
            rbo = dsq.tile([P, G * K], bf16, tag="rbo")
            nc.scalar.activation(out=rbo[:], in_=rb[:], func=Act.Square,
                                 accum_out=ACC_S[:, S_BW2:S_BW2 + 1])

        def epi_sums():
            u3 = used_t[K][:]
            unm = mp.tile([P, G * K], f32)
            nc.vector.tensor_scalar(out=unm[:], in0=u3, scalar1=-1.0,
                                    scalar2=1.0, op0=Alu.mult, op1=Alu.add)
            nc.vector.tensor_reduce(out=ACC_V[:, V_AMPS:V_AMPS + 1],
                                    in_=V[:, G * K:2 * G * K], axis=X, op=Alu.add)
            ua = mp.tile([P, G * K], f32)
            nc.vector._custom_dve(TENSOR_TENSOR_REDUCE, out=ua[:],
                                  in0=V[:, G * K:2 * G * K], in1=unm[:],
                                  s0=0.0, s1=1.0,
                                  accum_out=ACC_V[:, V_UMN:V_UMN + 1])
            nc.vector.tensor_reduce(out=ACC_V[:, V_UMD:V_UMD + 1], in_=unm[:],
                                    axis=X, op=Alu.add)
            nc.vector.tensor_reduce(out=ACC_V[:, V_MASK:V_MASK + 1], in_=M[:],
                                    axis=X, op=Alu.add)

        # ------------- DVE main loop: one fused op per tile --------------
        def huber_tile(t):
            d2 = dht.tile([P, F], bf16, tag="d2")
            nc.vector._custom_dve(HUBER_FULL2, out=d2[:], in0=pts[t][:],
                                  in1=tts[t][:], s0=1.0,
                                  accum_out=ACC_V[:, V_HF + t:V_HF + t + 1])

        match_prologue()
        for j in range(K):
            match_scan_step(j)
        huber_tile(0)
        Gt = epi_gather()
        huber_tile(1)
        epi_peaks(Gt)
        huber_tile(2)
        epi_sums()
        for t in range(3, NT):
            huber_tile(t)

        # ---------------- store both ACC tiles ----------------
        nc.sync.dma_start(out=out_d[:, 0:16], in_=ACC_S[:])
        nc.sync.dma_start(out=out_d[:, 16:32], in_=ACC_V[:])
    nc.compile()
    return nc


_NC_CACHE = None


def _get_nc():
    global _NC_CACHE
    if _NC_CACHE is None:
        _NC_CACHE = build_nc()
    return _NC_CACHE


def combine(parts):
    """parts: [n_cores, 32] float64 -> final scalar (python float)."""
    s = parts.sum(axis=0)
    SH = s[16 + V_HF:16 + V_HF + 8].sum()   # sum 2*huber(e)
    n_big = float(B) * F
    l_recon = 0.5 * SH / n_big
    l_sparse = s[16 + V_AMPS] / (B * K)
    l_bw = s[S_BW2] / (B * K)
    l_ap = s[S_AP] / B
    l_peaks = s[S_PK] / max(s[16 + V_MASK], 1.0)
    l_um = s[16 + V_UMN] / max(s[16 + V_UMD], 1.0)
    return (l_recon + 0.1 * l_sparse + 0.05 * l_bw + 0.5 * l_ap
            + 0.3 * l_peaks + 0.1 * l_um)


def run(inputs, **spmd_kwargs):
    nc = _get_nc()
    in_maps = []
    for c in range(N_CORES):
        lo, hi = c * BS, (c + 1) * BS
        in_maps.append({k: np.ascontiguousarray(v[lo:hi]) for k, v in inputs.items()})
    res = run_bass_kernel_spmd(nc, in_maps, list(range(N_CORES)), **spmd_kwargs)
    parts = np.stack([r["out"].astype(np.float64).sum(axis=0)
                      for r in res.results])
    return np.float32(combine(parts)), res


def kernel(**inputs):
    out, _ = run(inputs)
    return out


# revision 9
# speedup vs baseline: 1.4910x; 1.0023x over previous
"""DiffFOOOF loss on 8 NeuronCores — pure data parallelism over batch.

Each core processes B/8 = 1024 rows and emits per-column partial sums in
two [128, 16] accumulator tiles (one written by ScalarE accums, one by
DVE); the host does the final partition/core reduction in float64.

v3 engine split (from perfetto analysis of v1/v2):
  * DVE: e = p - t (fp32 in, bf16 out), then v = max(e,1) (4x bf16
    tensor_scalar) and m = max(-e, v) = max(|e|,1) (2x bf16 stt).
  * ScalarE: Square(e) and Square(m - 1), each with accum_out, one
    column per tile (the accumulate does the full free-dim reduce).
  * fp32 identity-matmul subtract on PE was tried and dropped: fp32
    matmuls run 4-pass (~600-1000ns per 512-col MM + LDW per call).
  * Greedy matching: no EPS prescale (used-flag scaled by 2^30 inside the
    per-step scalar_tensor_tensor), no iota tie-break (input data has no
    ties; verified host-side), inactive GT slots masked by adding
    (1-mask)*2^31 to the row-min before the is_equal.
  * 16 big loads issued round-robin over 4 HWDGE rings (sync/scalar/
    tensor/vector) so all DMA engines start within ~3us; all 16 tiles
    stay resident in SBUF (no buffer-recycling stalls).
"""

import numpy as np

import concourse.bass as bass
import concourse.tile as tile
from concourse import bacc, mybir
from concourse.bass_utils import run_bass_kernel_spmd

import concourse.dve_ops as dve_ops_mod
from concourse.dve_ops import DveOp, TENSOR_TENSOR_REDUCE
from concourse.dve_spec import Spec, Src0, Src1, C0, C1, sq, maxx, minn, Zero
from concourse.dve_spec import AluOp as DAlu

f32 = mybir.dt.float32
bf16 = mybir.dt.bfloat16
Alu = mybir.AluOpType
Act = mybir.ActivationFunctionType
X = mybir.AxisListType.X

N_CORES = 8
B, F, K = 8192, 2048, 6
BS = B // N_CORES        # rows per core
P = 128                  # partitions
NT = BS // P             # big [128, F] tiles per core
G = BS // P              # row-groups per partition for the small tensors
NCHUNK = 4               # 512-col matmul chunks per big tile
CW = F // NCHUNK
BIG_USED = float(2 ** 30)   # added (x used-flag) to mask used pred slots
BIG_ROW = float(2 ** 31)    # added to row-min for inactive GT slots

# ACC_S cols (ScalarE accum_out targets)
S_PK, S_AP, S_BW2 = 0, 1, 2
# ACC_V cols (DVE accum/reduce targets)
V_HF, V_AMPS, V_UMN, V_UMD, V_MASK = 0, 8, 9, 10, 11

SMALL_NAMES = ("cfs", "amps", "bws", "gt_cfs", "gt_amps", "gt_bws", "peak_mask")


def _register_huber_full2():
    """2-stream fused op: accum += sum(2*huber(p - t)) per partition.

    huber2(e) = e^2 - relu(|e|-1)^2 = m*(2a - m) with a=|e|, m=min(a,1):
    exactly the 8-stage DVE budget (SUB, SUB, MAX, MIN, ADD, SUB, MUL,
    accum-ADD), fp32 all the way.
    """
    for op in dve_ops_mod.OPS:
        if op.name == "HUBER_FULL2":
            return op

    def _ref(in0, in1, c0, c1, c2):
        a = np.abs(in0.astype(np.float32) - in1.astype(np.float32))
        m = np.minimum(a, c0)
        b = (m * (2.0 * a - m)).astype(np.float32)
        return b, b.reshape(b.shape[0], -1).sum(axis=-1, keepdims=True)

    e1 = Src0 - Src1
    e2 = Src1 - Src0
    a = maxx(e1, e2)
    m = minn(a, C0)
    op = DveOp(
        "HUBER_FULL2",
        Spec(body=m * (a + a - m),
             accum=DAlu.ADD, accum_init=Zero, reference=_ref),
        subdim=False,
        uops_sha={"v3": "37a2775ccf8d79d0", "v4": "c0009c8da2ade20d"},
    )
    dve_ops_mod.OPS.append(op)
    dve_ops_mod._SUB_OPCODE_FOR_NAME[op.name] = (
        dve_ops_mod._CUSTOM_DVE_ROW_BASE + len(dve_ops_mod.OPS) - 1)
    return op


HUBER_FULL2 = _register_huber_full2()


def build_nc():
    from contextlib import ExitStack

    nc = bacc.Bacc("TRN2", target_bir_lowering=False, debug=False,
                   num_devices=N_CORES)
    pred = nc.dram_tensor("pred_psd", [BS, F], f32, kind="ExternalInput")
    true = nc.dram_tensor("true_psd", [BS, F], f32, kind="ExternalInput")
    dr = {n: nc.dram_tensor(n, [BS, K], f32, kind="ExternalInput")
          for n in SMALL_NAMES}
    exponent = nc.dram_tensor("exponent", [BS, 1], f32, kind="ExternalInput")
    offset = nc.dram_tensor("offset", [BS, 1], f32, kind="ExternalInput")
    gt_exp = nc.dram_tensor("gt_exponent", [BS], f32, kind="ExternalInput")
    gt_off = nc.dram_tensor("gt_offset", [BS], f32, kind="ExternalInput")
    out_d = nc.dram_tensor("out", [P, 32], f32, kind="ExternalOutput")

    with tile.TileContext(nc) as tc, ExitStack() as ctx:
        sp = ctx.enter_context(tc.tile_pool(name="small", bufs=1))
        mp = ctx.enter_context(tc.tile_pool(name="match", bufs=1))
        pp = ctx.enter_context(tc.tile_pool(name="pred", bufs=NT))
        tp = ctx.enter_context(tc.tile_pool(name="true", bufs=NT))
        dsq = ctx.enter_context(tc.tile_pool(name="dsq", bufs=2))
        dht = ctx.enter_context(tc.tile_pool(name="dht", bufs=2))

        # ------------- big DMAs first, 4 rings in parallel --------------
        pts = [pp.tile([P, F], f32, tag="pt", name=f"pt{_t}") for _t in range(NT)]
        tts = [tp.tile([P, F], f32, tag="tt", name=f"tt{_t}") for _t in range(NT)]
        # only sync + scalar have HWDGE: pred on sync, true on scalar,
        # issued in tile order so early tiles' descriptors go first.
        # The last two tiles are split into column halves so the final
        # HUBER_FULL2 piece is half-size (shorter critical tail).
        H2 = F // 2
        for i in range(NT):
            if i < NT - 2:
                nc.sync.dma_start(out=pts[i][:], in_=pred[i * P:(i + 1) * P, :])
                nc.scalar.dma_start(out=tts[i][:], in_=true[i * P:(i + 1) * P, :])
            else:
                for h in range(2):
                    cs = slice(h * H2, (h + 1) * H2)
                    nc.sync.dma_start(out=pts[i][:, cs],
                                      in_=pred[i * P:(i + 1) * P, cs])
                    nc.scalar.dma_start(out=tts[i][:, cs],
                                        in_=true[i * P:(i + 1) * P, cs])

        # ------------- small tensors (gpsimd swdge ring) ----------------
        # row r = p*G + g; V col = v*(G*K) + g*K + i
        V = sp.tile([P, 3 * G * K], f32)
        GT = sp.tile([P, 3 * G * K], f32)
        M = sp.tile([P, G * K], f32)
        AUX = sp.tile([P, 4 * G], f32)
        # prologue-critical first: cfs, gt_cfs, peak_mask
        for v, name in ((0, "cfs"), (3, "gt_cfs")):
            dst = V if v < 3 else GT
            nc.gpsimd.dma_start(
                out=dst[:, (v % 3) * G * K:((v % 3) + 1) * G * K],
                in_=dr[name][:, :].rearrange("(p g) i -> p (g i)", g=G))
        nc.gpsimd.dma_start(
            out=M[:, :], in_=dr["peak_mask"][:, :].rearrange("(p g) j -> p (g j)", g=G))
        for v, name in ((1, "amps"), (2, "bws"), (4, "gt_amps"), (5, "gt_bws")):
            dst = V if v < 3 else GT
            nc.gpsimd.dma_start(
                out=dst[:, (v % 3) * G * K:((v % 3) + 1) * G * K],
                in_=dr[name][:, :].rearrange("(p g) i -> p (g i)", g=G))

        # AUX after the identity build (only needed by the epilogue)
        nc.gpsimd.dma_start(
            out=AUX[:, 0:G], in_=exponent[:, :].rearrange("(p g) o -> p (g o)", g=G))
        nc.gpsimd.dma_start(
            out=AUX[:, G:2 * G], in_=gt_exp[:].rearrange("(p g) -> p g", g=G))
        nc.gpsimd.dma_start(
            out=AUX[:, 2 * G:3 * G], in_=offset[:, :].rearrange("(p g) o -> p (g o)", g=G))
        nc.gpsimd.dma_start(
            out=AUX[:, 3 * G:4 * G], in_=gt_off[:].rearrange("(p g) -> p g", g=G))

        ACC_S = sp.tile([P, 16], f32)
        nc.vector.memset(ACC_S[:], 0.0)
        ACC_V = sp.tile([P, 16], f32)
        nc.vector.memset(ACC_V[:], 0.0)
        neg1 = sp.tile([P, 1], f32)
        nc.vector.memset(neg1[:], -1.0)

        # ------------- matching tiles ------------------------------------
        V4 = V[:].rearrange("p (v g i) -> p g v i", v=3, i=K)
        M3 = M[:].rearrange("p (g j) -> p g j", j=K)
        gtp3 = GT[:, 0:G * K].rearrange("p (g j) -> p g j", j=K)
        cfs3 = V[:, 0:G * K].rearrange("p (g i) -> p g i", i=K)

        dist = mp.tile([P, G * K * K], f32)   # col = g*36 + j*6 + i
        dabs = mp.tile([P, G * K * K], f32)
        dist4 = dist[:].rearrange("p (g j i) -> p g j i", j=K, i=K)
        dabs4 = dabs[:].rearrange("p (g j i) -> p g j i", j=K, i=K)
        NEG = mp.tile([P, G * K], f32)        # (1-mask)*2^31, col = g*6 + j
        NEG3 = NEG[:].rearrange("p (g j) -> p g j", j=K)
        H = mp.tile([P, G * K * K], f32)      # one-hot hits, col = g*36+j*6+i
        H4 = H[:].rearrange("p (g j i) -> p g j i", j=K, i=K)
        used_t = [mp.tile([P, G * K], f32, tag=f"used{j}", name=f"used{j}")
                  for j in range(K + 1)]

        def match_prologue():
            nc.vector.tensor_tensor(
                out=dist4,
                in0=gtp3.to_broadcast([P, G, K, K]),
                in1=cfs3.unsqueeze(2).to_broadcast([P, G, K, K]),
                op=Alu.subtract)
            nc.vector.scalar_tensor_tensor(out=dabs4, in0=dist4, scalar=-1.0,
                                           in1=dist4, op0=Alu.mult, op1=Alu.max)
            nc.vector.tensor_scalar(out=NEG[:], in0=M[:], scalar1=-BIG_ROW,
                                    scalar2=BIG_ROW, op0=Alu.mult, op1=Alu.add)
            nc.vector.memset(used_t[0][:], 0.0)

        def match_scan_step(j):
            u3 = used_t[j][:].rearrange("p (g i) -> p g i", i=K)
            dm = mp.tile([P, G * K], f32, tag="dm")
            dm3 = dm[:].rearrange("p (g i) -> p g i", i=K)
            nc.vector.scalar_tensor_tensor(out=dm3, in0=u3, scalar=BIG_USED,
                                           in1=dabs4[:, :, j, :],
                                           op0=Alu.mult, op1=Alu.add)
            mv = mp.tile([P, G], f32, tag="mv")
            nc.vector.tensor_reduce(out=mv[:], in_=dm3, axis=X, op=Alu.min)
            mvp = mp.tile([P, G], f32, tag="mvp")
            nc.vector.tensor_tensor(out=mvp[:], in0=mv[:], in1=NEG3[:, :, j],
                                    op=Alu.add)
            hj = H4[:, :, j, :]
            nc.vector.tensor_tensor(out=hj, in0=dm3,
                                    in1=mvp[:].to_broadcast([P, G, K]),
                                    op=Alu.is_equal)
            un3 = used_t[j + 1][:].rearrange("p (g i) -> p g i", i=K)
            nc.vector.tensor_tensor(out=un3, in0=u3, in1=hj, op=Alu.add)

        def epi_gather():
            # Gt[p,v,g,j] = sum_i H[p,g,j,i] * V[p,v,g,i]
            gm = mp.tile([P, 3 * G * K * K], f32)
            gm5 = gm[:].rearrange("p (v g j i) -> p v g j i", v=3, j=K, i=K)
            Vv = V[:].rearrange("p (v g i) -> p v g i", v=3, i=K)
            nc.vector.tensor_tensor(
                out=gm5,
                in0=Vv.unsqueeze(3).to_broadcast([P, 3, G, K, K]),
                in1=H4.unsqueeze(1).to_broadcast([P, 3, G, K, K]),
                op=Alu.mult)
            Gt = mp.tile([P, 3 * G * K], f32)
            Gt4 = Gt[:].rearrange("p (v g j) -> p v g j", v=3, j=K)
            nc.vector.tensor_reduce(out=Gt4, in_=gm5, axis=X, op=Alu.add)
            return Gt

        def epi_peaks(Gt):
            D = mp.tile([P, 3 * G * K], f32)
            nc.vector.tensor_tensor(out=D[:], in0=Gt[:], in1=GT[:],
                                    op=Alu.subtract)
            Dm = mp.tile([P, 3 * G * K], f32)
            nc.vector.tensor_tensor(
                out=Dm[:].rearrange("p (v gj) -> p v gj", v=3),
                in0=D[:].rearrange("p (v gj) -> p v gj", v=3),
                in1=M[:].unsqueeze(1).to_broadcast([P, 3, G * K]),
                op=Alu.mult)
            dmp = dsq.tile([P, 3 * G * K], bf16, tag="dmp")
            nc.scalar.activation(out=dmp[:], in_=Dm[:], func=Act.Square,
                                 accum_out=ACC_S[:, S_PK:S_PK + 1])
            # aperiodic: pack [dE | dO] then one Square-accum
            DEO = mp.tile([P, 2 * G], f32)
            nc.vector.tensor_tensor(out=DEO[:, 0:G], in0=AUX[:, 0:G],
                                    in1=AUX[:, G:2 * G], op=Alu.subtract)
            nc.vector.tensor_tensor(out=DEO[:, G:2 * G], in0=AUX[:, 2 * G:3 * G],
                                    in1=AUX[:, 3 * G:4 * G], op=Alu.subtract)
            deo = dsq.tile([P, 2 * G], bf16, tag="deo")
            nc.scalar.activation(out=deo[:], in_=DEO[:], func=Act.Square,
                                 accum_out=ACC_S[:, S_AP:S_AP + 1])
            # bw excess
            rb = mp.tile([P, G * K], f32)
            nc.vector.tensor_scalar(out=rb[:], in0=V[:, 2 * G * K:3 * G * K],
                                    scalar1=4.0, scalar2=0.0,
                                    op0=Alu.subtract, op1=Alu.max)
            rbo = dsq.tile([P, G * K], bf16, tag="rbo")
            nc.scalar.activation(out=rbo[:], in_=rb[:], func=Act.Square,
                                 accum_out=ACC_S[:, S_BW2:S_BW2 + 1])

        def epi_sums():
            u3 = used_t[K][:]
            unm = mp.tile([P, G * K], f32)
            nc.vector.tensor_scalar(out=unm[:], in0=u3, scalar1=-1.0,
                                    scalar2=1.0, op0=Alu.mult, op1=Alu.add)
            nc.vector.tensor_reduce(out=ACC_V[:, V_AMPS:V_AMPS + 1],
                                    in_=V[:, G * K:2 * G * K], axis=X, op=Alu.add)
            ua = mp.tile([P, G * K], f32)
            nc.vector._custom_dve(TENSOR_TENSOR_REDUCE, out=ua[:],
                                  in0=V[:, G * K:2 * G * K], in1=unm[:],
                                  s0=0.0, s1=1.0,
                                  accum_out=ACC_V[:, V_UMN:V_UMN + 1])
            nc.vector.tensor_reduce(out=ACC_V[:, V_UMD:V_UMD + 1], in_=unm[:],
                                    axis=X, op=Alu.add)
            nc.vector.tensor_reduce(out=ACC_V[:, V_MASK:V_MASK + 1], in_=M[:],
                                    axis=X, op=Alu.add)

        # ------------- DVE main loop: one fused op per tile --------------
        def huber_tile(t):
            if t < NT - 2:
                d2 = dht.tile([P, F], bf16, tag="d2")
                nc.vector._custom_dve(HUBER_FULL2, out=d2[:], in0=pts[t][:],
                                      in1=tts[t][:], s0=1.0,
                                      accum_out=ACC_V[:, V_HF + t:V_HF + t + 1])
            else:
                # split tiles: two half-width ops, second accum col at
                # 12 (t=6) / 13 (t=7)
                cols = (V_HF + t, 12 + (t - (NT - 2)))
                for h in range(2):
                    cs = slice(h * H2, (h + 1) * H2)
                    d2 = dht.tile([P, H2], bf16, tag="d2h")
                    nc.vector._custom_dve(HUBER_FULL2, out=d2[:],
                                          in0=pts[t][:, cs], in1=tts[t][:, cs],
                                          s0=1.0,
                                          accum_out=ACC_V[:, cols[h]:cols[h] + 1])

        match_prologue()
        for j in range(K):
            match_scan_step(j)
        huber_tile(0)
        Gt = epi_gather()
        huber_tile(1)
        epi_peaks(Gt)
        huber_tile(2)
        epi_sums()
        for t in range(3, NT):
            huber_tile(t)

        # ---------------- store both ACC tiles ----------------
        nc.sync.dma_start(out=out_d[:, 0:16], in_=ACC_S[:])
        nc.sync.dma_start(out=out_d[:, 16:32], in_=ACC_V[:])
    nc.compile()
    return nc


_NC_CACHE = None


def _get_nc():
    global _NC_CACHE
    if _NC_CACHE is None:
        _NC_CACHE = build_nc()
    return _NC_CACHE


def combine(parts):
    """parts: [n_cores, 32] float64 -> final scalar (python float)."""
    s = parts.sum(axis=0)
    SH = s[16 + V_HF:16 + V_HF + 8].sum() + s[16 + 12] + s[16 + 13]
    n_big = float(B) * F
    l_recon = 0.5 * SH / n_big
    l_sparse = s[16 + V_AMPS] / (B * K)
    l_bw = s[S_BW2] / (B * K)
    l_ap = s[S_AP] / B
    l_peaks = s[S_PK] / max(s[16 + V_MASK], 1.0)
    l_um = s[16 + V_UMN] / max(s[16 + V_UMD], 1.0)
    return (l_recon + 0.1 * l_sparse + 0.05 * l_bw + 0.5 * l_ap
            + 0.3 * l_peaks + 0.1 * l_um)


def run(inputs, **spmd_kwargs):
    nc = _get_nc()
    in_maps = []
    for c in range(N_CORES):
        lo, hi = c * BS, (c + 1) * BS
        in_maps.append({k: np.ascontiguousarray(v[lo:hi]) for k, v in inputs.items()})
    res = run_bass_kernel_spmd(nc, in_maps, list(range(N_CORES)), **spmd_kwargs)
    parts = np.stack([r["out"].astype(np.float64).sum(axis=0)
                      for r in res.results])
    return np.float32(combine(parts)), res


def kernel(**inputs):
    out, _ = run(inputs)
    return out
